# revision 1
# baseline (speedup 1.0000x reference)
"""Multi-head self-attention (B=2, T=2048, C=1024, H=16) on 8 TRN2 NeuronCores.

Sharding: tensor-parallel over heads. Core m owns heads (2m, 2m+1):
  - computes qkv^T = (Wqkv_shard^T) @ x^T for its 2 heads (contraction-major
    layouts; host pre-transposes x so no on-chip transposes of x are needed)
  - causal attention for its 2 heads (both batches), flash-style with
    blockwise exp (no max-subtraction: scores are O(1) here) and a
    ones-column matmul that produces the softmax denominator for free
  - partial output projection partial_m = values_m @ Wo[rows of heads m]
Host sums the 8 partials and adds biases bo.
"""

import numpy as np

import concourse.bass as bass
import concourse.bacc as bacc
import concourse.mybir as mybir
import concourse.tile as tile
from concourse.bass_utils import run_bass_kernel_spmd

B, T, C = 2, 2048, 1024
H, HS = 16, 64
N_CORES = 8
HPC = H // N_CORES            # heads per core = 2
R = B * T                      # 4096 rows total
IC_W = 512                     # i-chunk width (query cols per block)
JT_W = 128                     # j-tile width (key rows per block)
N_IC = T // IC_W               # 4 i-chunks per batch
N_JT = T // JT_W               # 16 j-tiles per batch
F32 = mybir.dt.float32
F32R = mybir.dt.float32r

# compute dtype for matmuls: float32r runs the PE at full rate (1 cyc/row for
# free-dim >= 256) with near-fp32 storage; plain float32 is 4x slower.
USE_F32R = True


def _mm(d):
    return d


def _build(causal: bool, reps: int = 1, stage: int = 5):
    nc = bacc.Bacc("TRN2", target_bir_lowering=False, debug=False,
                   num_devices=N_CORES)

    xt_d = nc.dram_tensor("xt", [C, R], F32R, kind="ExternalInput")
    w3_d = nc.dram_tensor("w3", [C, 3 * 128], F32R, kind="ExternalInput")
    b3_d = nc.dram_tensor("b3", [3, 128, 1], F32, kind="ExternalInput")
    wo_d = nc.dram_tensor("wo", [128, C], F32R, kind="ExternalInput")
    msk_d = nc.dram_tensor("msk", [4, 128, IC_W], F32R, kind="ExternalInput")
    idn_d = nc.dram_tensor("idn", [128, 64], F32R, kind="ExternalInput")
    one_d = nc.dram_tensor("ones", [128, 64], F32R, kind="ExternalInput")
    out_d = nc.dram_tensor("part", [R, C], F32, kind="ExternalOutput")

    with tile.TileContext(nc) as tc:
        with (
            tc.tile_pool(name="const", bufs=1) as cpool,
            tc.tile_pool(name="qkv", bufs=1) as qkvpool,
            tc.tile_pool(name="xt", bufs=16) as xtpool,
            tc.tile_pool(name="pt", bufs=6) as ptpool,
            tc.tile_pool(name="small", bufs=4) as smallpool,
            tc.tile_pool(name="ot", bufs=2) as otpool,
            tc.tile_pool(name="osb", bufs=4) as opool,
            tc.tile_pool(name="ps_mm", bufs=2, space="PSUM") as ps_mm,
            tc.tile_pool(name="ps_s", bufs=3, space="PSUM") as ps_s,
            tc.tile_pool(name="ps_o", bufs=2, space="PSUM") as ps_o,
            tc.tile_pool(name="ps_cs", bufs=1, space="PSUM") as ps_cs,
        ):
            for rep in range(reps):
                # ---- constants ----
                w3_sb = []
                for ct in range(8):
                    t_ = cpool.tile([128, 384], F32R, tag=f"w3_{ct}")
                    nc.sync.dma_start(t_[:], w3_d.ap()[128 * ct:128 * (ct + 1), :])
                    w3_sb.append(t_)
                bias_sb = []
                for n in range(3):
                    t_ = cpool.tile([128, 1], F32, tag=f"b3_{n}")
                    nc.sync.dma_start(t_[:], b3_d.ap()[n])
                    bias_sb.append(t_)
                wo_sb = cpool.tile([128, C], F32R, tag="wo")
                nc.sync.dma_start(wo_sb[:], wo_d.ap()[:])
                msk_sb = []
                for r_ in range(4):
                    t_ = cpool.tile([128, IC_W], F32R, tag=f"msk_{r_}")
                    nc.sync.dma_start(t_[:], msk_d.ap()[r_])
                    msk_sb.append(t_)
                idn_sb = cpool.tile([128, 64], F32R, tag="idn")
                nc.sync.dma_start(idn_sb[:], idn_d.ap()[:])
                ones_sb = cpool.tile([128, 64], F32R, tag="ones_sb")
                nc.sync.dma_start(ones_sb[:], one_d.ap()[:])


                # persistent qkv^T (transposed layouts, heads packed 2-up)
                qt2b = [qkvpool.tile([128, T], F32R, tag=f"qt2_{b_}", name=f"qt2_{b_}")
                        for b_ in range(B)]
                kt2b = [qkvpool.tile([128, T], F32R, tag=f"kt2_{b_}", name=f"kt2_{b_}")
                        for b_ in range(B)]
                vt2b = [qkvpool.tile([128, T], F32R, tag=f"vt2_{b_}", name=f"vt2_{b_}")
                        for b_ in range(B)]
                # v in natural [key, dim] layout per (b, h): [128, 16*64]
                vn_sb = {}
                for b in range(B):
                    for h in range(HPC):
                        vn_sb[(b, h)] = qkvpool.tile([128, N_JT * (HS + 1)], F32R,
                                                     name=f"vn_{rep}_{b}_{h}",
                                                     tag=f"vn_{b}_{h}")

                for b in range(B):
                    # ---- QKV projection for this batch's 4 i-chunks ----
                    for icl in range(4):
                        i0 = IC_W * (4 * b + icl)
                        xts = []
                        for ct in range(8):
                            t_ = xtpool.tile([128, IC_W], F32R)
                            nc.sync.dma_start(
                                t_[:], xt_d.ap()[128 * ct:128 * (ct + 1),
                                                 i0:i0 + IC_W])
                            xts.append(t_)
                        for n, dst in enumerate((qt2b[b], kt2b[b], vt2b[b])):
                            ps = ps_mm.tile([128, IC_W], F32, tag="mm")
                            for ct in range(8):
                                nc.tensor.matmul(
                                    ps[:],
                                    _mm(w3_sb[ct][:, 128 * n:128 * (n + 1)]),
                                    _mm(xts[ct][:]),
                                    start=(ct == 0), stop=(ct == 7))
                            nc.vector.tensor_scalar_add(
                                dst[:, IC_W * icl:IC_W * (icl + 1)], ps[:],
                                bias_sb[n][:])

                    # ---- v natural layout (transpose v^T tiles) ----
                    for h in range(HPC if stage >= 2 else 0):
                        for jt in range(N_JT):
                            j0 = JT_W * jt
                            psv = ps_mm.tile([128, IC_W], F32R, tag="mm")
                            nc.tensor.transpose(
                                psv[:, 0:HS],
                                vt2b[b][64 * h:64 * (h + 1), j0:j0 + JT_W],
                                idn_sb[64 * h:64 * (h + 1), 0:64])
                            nc.vector.tensor_copy(
                                vn_sb[(b, h)][:, 65 * jt:65 * jt + HS],
                                psv[:, 0:HS])
                            nc.vector.tensor_copy(
                                vn_sb[(b, h)][:, 65 * jt + HS:65 * (jt + 1)],
                                ones_sb[:, 0:1])

                    # ---- attention + projection per i-chunk ----
                    for icl in range(4 if stage >= 3 else 0):
                        i0 = IC_W * icl
                        njt = 4 * icl + 4 if causal else N_JT
                        pso = [ps_o.tile([128, IC_W], F32, tag="o", name=f"pso_{h_}")
                               for h_ in range(HPC)]

                        for jt in range(njt):
                            j0 = JT_W * jt
                            for h in range(HPC):
                                h0 = 64 * h
                                pss = ps_s.tile([128, IC_W], F32, tag="s")
                                nc.tensor.matmul(
                                    pss[:],
                                    _mm(kt2b[b][h0:h0 + 64, j0:j0 + JT_W]),
                                    _mm(qt2b[b][h0:h0 + 64, i0:i0 + IC_W]),
                                    start=True, stop=True,
                                    tile_position=(h0, 0))
                                pt = ptpool.tile([128, IC_W], F32R, tag="pt")
                                nc.scalar.activation(
                                    pt[:], pss[:],
                                    mybir.ActivationFunctionType.Exp)
                                if causal:
                                    r_ = jt - 4 * icl
                                    if r_ >= 0:
                                        nc.gpsimd.tensor_mul(
                                            pt[:], pt[:], msk_sb[r_][:])
                                nc.tensor.matmul(
                                    pso[h][0:65, :],
                                    _mm(vn_sb[(b, h)][:, 65 * jt:65 * (jt + 1)]),
                                    _mm(pt[:]),
                                    start=(jt == 0), stop=(jt == njt - 1),
                                    tile_position=(0, 0), skip_group_check=True)
                        # normalize -> ot [128, 512] (h0 rows 0:64, h1 rows 64:128)
                        ot = otpool.tile([128, IC_W], F32R, tag="ot")
                        for h in range(HPC if stage >= 4 else 0):
                            csr = smallpool.tile([1, IC_W], F32R, tag="csr")
                            nc.scalar.activation(
                                csr[:], pso[h][64:65, :],
                                mybir.ActivationFunctionType.Copy)
                            pscb = ps_cs.tile([64, IC_W], F32, tag="cs")
                            nc.tensor.matmul(
                                pscb[:], _mm(ones_sb[0:1, :]), _mm(csr[:]),
                                start=True, stop=True)
                            rcb = smallpool.tile([64, IC_W], F32, tag="rcb")
                            nc.vector.reciprocal(rcb[:], pscb[:])
                            nc.vector.tensor_mul(
                                ot[64 * h:64 * (h + 1), :], pso[h][0:64, :],
                                rcb[:])
                        # projection: partial[i0:i0+512, :] = ot^T @ wo
                        for it in range(4 if stage >= 5 else 0):
                            osb = opool.tile([128, C], F32, tag="osb")
                            for oc in range(2):
                                psp = ps_mm.tile([128, IC_W], F32, tag="mm")
                                nc.tensor.matmul(
                                    psp[:],
                                    _mm(ot[:, 128 * it:128 * (it + 1)]),
                                    _mm(wo_sb[:, IC_W * oc:IC_W * (oc + 1)]),
                                    start=True, stop=True)
                                nc.vector.tensor_copy(
                                    osb[:, IC_W * oc:IC_W * (oc + 1)], psp[:])
                            r0 = T * b + i0 + 128 * it
                            nc.sync.dma_start(
                                out_d.ap()[r0:r0 + 128, :], osb[:])
    nc.compile()
    return nc


_PROGS = {}


def _get_prog(causal: bool, reps: int = 1, stage: int = 5):
    key = (causal, reps, stage)
    if key not in _PROGS:
        _PROGS[key] = _build(causal, reps, stage)
    return _PROGS[key]


def _prep_inputs(x, Wqkv, bqkv, Wo):
    """Per-core input maps (host-side sharding)."""
    x = np.asarray(x, dtype=np.float32)
    Wqkv = np.asarray(Wqkv, dtype=np.float32)
    bqkv = np.asarray(bqkv, dtype=np.float32)
    Wo = np.asarray(Wo, dtype=np.float32)

    xt = np.ascontiguousarray(x.reshape(R, C).T)  # [C, R]

    # causal mask tiles for the 4 diagonal block offsets
    jl = np.arange(JT_W)[:, None]
    il = np.arange(IC_W)[None, :]
    msk = np.stack([(JT_W * r_ + jl <= il) for r_ in range(4)]).astype(np.float32)
    idn = np.tile(np.eye(64, dtype=np.float32), (2, 1))

    in_maps = []
    scale = 1.0 / np.sqrt(np.float32(HS))
    for m in range(N_CORES):
        h0, h1 = HPC * m, HPC * m + 1
        cols = {}
        for name, off, sc in (("q", 0, scale), ("k", HS, 1.0), ("v", 2 * HS, 1.0)):
            blk = [Wqkv[:, 192 * h + off:192 * h + off + HS] * sc
                   for h in (h0, h1)]
            bb = [bqkv[192 * h + off:192 * h + off + HS] * sc for h in (h0, h1)]
            cols[name] = (np.concatenate(blk, axis=1),
                          np.concatenate(bb))
        w3 = np.concatenate([cols["q"][0], cols["k"][0], cols["v"][0]], axis=1)
        b3 = np.stack([cols["q"][1], cols["k"][1], cols["v"][1]])[..., None]
        wo = Wo[128 * m:128 * (m + 1), :]
        in_maps.append({
            "xt": np.ascontiguousarray(xt),
            "w3": np.ascontiguousarray(w3.astype(np.float32)),
            "b3": np.ascontiguousarray(b3.astype(np.float32)),
            "wo": np.ascontiguousarray(wo.astype(np.float32)),
            "msk": msk,
            "idn": idn,
            "ones": np.ones((128, 64), dtype=np.float32),
        })
    return in_maps


class _Runner:
    """Cached shard_map runner for the SPMD NEFF (avoids re-jit per call)."""

    def __init__(self, nc):
        import jax
        from jax.sharding import Mesh, PartitionSpec
        from jax.experimental.shard_map import shard_map
        from concourse import bass2jax

        bass2jax.install_neuronx_cc_hook()

        part_name = (nc.partition_id_tensor.name
                     if nc.partition_id_tensor else None)
        in_names, out_names, out_avals, zero_outs = [], [], [], []
        for alloc in nc.m.functions[0].allocations:
            if not isinstance(alloc, mybir.MemoryLocationSet):
                continue
            name = alloc.memorylocations[0].name
            if alloc.kind == "ExternalInput":
                if name != part_name:
                    in_names.append(name)
            elif alloc.kind == "ExternalOutput":
                out_names.append(name)
                shape = tuple(alloc.tensor_shape)
                dtype = mybir.dt.np(alloc.dtype)
                out_avals.append(jax.core.ShapedArray(shape, dtype))
                zero_outs.append(np.zeros(shape, dtype))
        self.in_names, self.out_names = in_names, out_names
        self.zero_outs = zero_outs
        n_params, n_outs = len(in_names), len(out_names)
        all_in_names = tuple(in_names) + tuple(out_names)
        if part_name is not None:
            all_in_names = all_in_names + (part_name,)

        def _exec(args, outs):
            operands = list(args) + list(outs)
            if part_name is not None:
                operands.append(bass2jax.partition_id_tensor())
            return bass2jax._bass_exec_p.bind(
                *operands,
                out_avals=tuple(out_avals),
                in_names=all_in_names,
                out_names=tuple(out_names),
                lowering_input_output_aliases=(),
                sim_require_finite=True,
                sim_require_nnan=True,
                nc=nc)

        def _body(*args):
            ins, outs = args[:n_params], list(args[n_params:])
            return tuple(_exec(ins, outs))

        devices = jax.devices()[:N_CORES]
        mesh = Mesh(np.asarray(devices), ("core",))
        donate = tuple(range(n_params, n_params + n_outs))
        self._fn = jax.jit(
            shard_map(_body, mesh=mesh,
                      in_specs=(PartitionSpec("core"),) * (n_params + n_outs),
                      out_specs=(PartitionSpec("core"),) * n_outs,
                      check_rep=False),
            donate_argnums=donate, keep_unused=True)

    def __call__(self, in_maps):
        concat_in = [
            np.concatenate([in_maps[c][k] for c in range(N_CORES)], axis=0)
            for k in self.in_names]
        concat_zero = [
            np.zeros((N_CORES * z.shape[0], *z.shape[1:]), z.dtype)
            for z in self.zero_outs]
        out = self._fn(*concat_in, *concat_zero)
        return [
            {k: np.asarray(out[i]).reshape(N_CORES, *self.zero_outs[i].shape)[c]
             for i, k in enumerate(self.out_names)}
            for c in range(N_CORES)]


_RUNNERS = {}


def _get_runner(causal: bool, reps: int = 1):
    key = (causal, reps)
    if key not in _RUNNERS:
        _RUNNERS[key] = _Runner(_get_prog(causal, reps))
    return _RUNNERS[key]


def kernel(x, Wqkv, bqkv, Wo, bo, mask):
    causal = bool(np.asarray(mask).item()) if not isinstance(mask, (int, bool)) else bool(mask)
    runner = _get_runner(causal)
    in_maps = _prep_inputs(x, Wqkv, bqkv, Wo)
    results = runner(in_maps)
    acc = np.zeros((R, C), dtype=np.float32)
    for m in range(N_CORES):
        acc += results[m]["part"]
    acc += np.asarray(bo, dtype=np.float32)[None, :]
    return acc.reshape(B, T, C)



# revision 13
# speedup vs baseline: 6.0286x; 6.0286x over previous
"""Multi-head self-attention (B=2, T=2048, C=1024, H=16) on 8 TRN2 NeuronCores.

Sharding: hybrid batch x head-group. Core c owns batch b = c//4 and heads
4g..4g+3 where g = c%4 (two head-pairs). Per core:
  - QKV projection for its batch rows and its 4 heads (bf16 matmuls,
    contraction-major x^T layout, biases folded in via tensor_scalar_add)
  - causal attention for its 4 heads, flash-style blockwise exp with no
    max-subtraction (scores are O(1)); causal mask applied as a -1e9 bias
    accumulated into the scores PSUM via an identity matmul, so exp gives
    exact zeros (no separate mask multiply)
  - softmax denominator via a ones-column in the value matrix (row 64 of
    the AV accumulation); per-i-chunk normalization with
    reciprocal_approx_fast + a selector-matmul broadcast
  - partial output projection partial = values @ Wo[rows of its heads]
Host sums the 4 partials per batch and adds bo.
"""

import numpy as np
import ml_dtypes

import concourse.bass as bass
import concourse.bacc as bacc
import concourse.mybir as mybir
import concourse.tile as tile
from concourse.bass_utils import run_bass_kernel_spmd

B, T, C = 2, 2048, 1024
H, HS = 16, 64
N_CORES = 8
IC_W = 512                     # i-chunk width (query cols per block)
N_IC = T // IC_W               # 4 i-chunks
JT = 128                       # j-tile width (key rows per block)
N_JT = T // JT                 # 16 j-tiles
BF16 = mybir.dt.bfloat16
F32 = mybir.dt.float32
NPBF = ml_dtypes.bfloat16


def _build(causal: bool):
    nc = bacc.Bacc("TRN2", target_bir_lowering=False, debug=False,
                   num_devices=N_CORES)

    xt_d = nc.dram_tensor("xt", [C, T], BF16, kind="ExternalInput")
    w3_d = nc.dram_tensor("w3", [C, 768], BF16, kind="ExternalInput")
    b3_d = nc.dram_tensor("b3", [6, 128, 1], F32, kind="ExternalInput")
    wo_d = nc.dram_tensor("wo", [256, C], BF16, kind="ExternalInput")
    msk_d = nc.dram_tensor("msk", [4, 128, IC_W], BF16, kind="ExternalInput")
    idn_d = nc.dram_tensor("idn", [128, 128], BF16, kind="ExternalInput")
    one_d = nc.dram_tensor("ones", [1, 64], BF16, kind="ExternalInput")
    vni_d = nc.dram_tensor("vni", [128, 65 * N_JT], BF16, kind="ExternalInput")
    out_d = nc.dram_tensor("part", [T, C], BF16, kind="ExternalOutput")

    with tile.TileContext(nc) as tc:
        with (
            tc.tile_pool(name="const", bufs=1) as cpool,
            tc.tile_pool(name="xt", bufs=2) as xtpool,
            tc.tile_pool(name="pt", bufs=4) as ptpool,
            tc.tile_pool(name="otu", bufs=4) as otupool,
            tc.tile_pool(name="ot", bufs=4) as otpool,
            tc.tile_pool(name="csr", bufs=4) as csrpool,
            tc.tile_pool(name="rt", bufs=2) as rtpool,
            tc.tile_pool(name="osb", bufs=2) as opool,
            tc.tile_pool(name="ps_s", bufs=2, space="PSUM") as ps_s,
            tc.tile_pool(name="ps_o", bufs=2, space="PSUM") as ps_o,
            tc.tile_pool(name="ps_m", bufs=2, space="PSUM") as ps_m,
        ):
            # ---- constants ----
            w3_sb = []
            for ct in range(8):
                t_ = cpool.tile([128, 768], BF16, tag=f"w3_{ct}", name=f"w3_{ct}")
                nc.sync.dma_start(t_[:], w3_d.ap()[128 * ct:128 * (ct + 1), :])
                w3_sb.append(t_)
            b3_sb = []
            for i in range(6):
                t_ = cpool.tile([128, 1], F32, tag=f"b3_{i}", name=f"b3_{i}")
                nc.sync.dma_start(t_[:], b3_d.ap()[i])
                b3_sb.append(t_)
            wo_sb = []
            for p in range(2):
                t_ = cpool.tile([128, C], BF16, tag=f"wo_{p}", name=f"wo_{p}")
                nc.sync.dma_start(t_[:], wo_d.ap()[128 * p:128 * (p + 1), :])
                wo_sb.append(t_)
            msk_sb = []
            for r in range(4):
                t_ = cpool.tile([128, IC_W], BF16, tag=f"msk_{r}", name=f"msk_{r}")
                nc.sync.dma_start(t_[:], msk_d.ap()[r])
                msk_sb.append(t_)
            idn_sb = cpool.tile([128, 128], BF16, tag="idn")
            nc.sync.dma_start(idn_sb[:], idn_d.ap()[:])
            onesr = cpool.tile([1, 64], BF16, tag="onesr")
            nc.sync.dma_start(onesr[:], one_d.ap()[:])
            vn_sb = []
            for h in range(4):
                t_ = cpool.tile([128, 65 * N_JT], BF16, tag=f"vn_{h}",
                                name=f"vn_{h}")
                nc.sync.dma_start(t_[:], vni_d.ap()[:])
                vn_sb.append(t_)

            qt2 = [cpool.tile([128, T], BF16, tag=f"qt2_{p}", name=f"qt2_{p}")
                   for p in range(2)]
            kt2 = [cpool.tile([128, T], BF16, tag=f"kt2_{p}", name=f"kt2_{p}")
                   for p in range(2)]
            vt2 = [cpool.tile([128, T], BF16, tag=f"vt2_{p}", name=f"vt2_{p}")
                   for p in range(2)]


            for icl in range(N_IC):
                i0 = IC_W * icl
                # ---- x^T chunk load ----
                xts = []
                for ct in range(8):
                    t_ = xtpool.tile([128, IC_W], BF16, tag=f"x{ct}",
                                     name=f"x{ct}")
                    nc.sync.dma_start(
                        t_[:], xt_d.ap()[128 * ct:128 * (ct + 1), i0:i0 + IC_W])
                    xts.append(t_)
                # ---- QKV projection for this chunk ----
                for p in range(2):
                    for n, dsts in enumerate((qt2, kt2, vt2)):
                        c0 = 384 * p + 128 * n
                        ps = ps_m.tile([128, IC_W], F32, tag="m", name="ps_qkv")
                        for ct in range(8):
                            nc.tensor.matmul(
                                ps[:], w3_sb[ct][:, c0:c0 + 128], xts[ct][:],
                                start=(ct == 0), stop=(ct == 7))
                        nc.vector.tensor_scalar_add(
                            dsts[p][:, i0:i0 + IC_W], ps[:], b3_sb[3 * p + n][:])
                # ---- v natural layout (transpose this chunk's j-tiles) ----
                for p in range(2):
                    for jl in range(4):
                        jt = 4 * icl + jl
                        j0 = JT * jt
                        psv = ps_m.tile([128, JT], BF16, tag="m", name="psv")
                        nc.tensor.transpose(
                            psv[:, 0:JT], vt2[p][:, j0:j0 + JT], idn_sb[:])
                        nc.vector.tensor_copy(
                            vn_sb[2 * p][:, 65 * jt:65 * jt + 64], psv[:, 0:64])
                        nc.vector.tensor_copy(
                            vn_sb[2 * p + 1][:, 65 * jt:65 * jt + 64],
                            psv[:, 64:128])
                # ---- attention for this chunk ----
                njt = 4 * (icl + 1) if causal else N_JT
                otus = []
                for p in range(2):
                    pso = [ps_o.tile([128, IC_W], F32, tag="o",
                                     name=f"pso{h_}") for h_ in range(2)]
                    for jtp in range(njt // 2):
                        pts = []
                        for h in range(2):
                            st = ps_s.tile([128, 2 * IC_W], F32, tag="s",
                                           name="st")
                            for half in range(2):
                                jt = 2 * jtp + half
                                j0 = JT * jt
                                c0 = IC_W * half
                                first = True
                                if causal and jt >= 4 * icl:
                                    r_ = jt - 4 * icl
                                    nc.tensor.matmul(
                                        st[:, c0:c0 + IC_W], idn_sb[:],
                                        msk_sb[r_][:],
                                        start=True, stop=False)
                                    first = False
                                nc.tensor.matmul(
                                    st[:, c0:c0 + IC_W],
                                    kt2[p][64 * h:64 * h + 64, j0:j0 + JT],
                                    qt2[p][64 * h:64 * h + 64, i0:i0 + IC_W],
                                    start=first, stop=True,
                                    tile_position=(64 * h, 0))
                            pt = ptpool.tile([128, 2 * IC_W], BF16, tag="pt",
                                             name="pt")
                            nc.scalar.activation(
                                pt[:], st[:], mybir.ActivationFunctionType.Exp)
                            pts.append(pt)
                        for h in range(2):
                            for half in range(2):
                                jt = 2 * jtp + half
                                nc.tensor.matmul(
                                    pso[h][0:65, :],
                                    vn_sb[2 * p + h][:, 65 * jt:65 * jt + 65],
                                    pts[h][:, IC_W * half:IC_W * (half + 1)],
                                    start=(jt == 0), stop=(jt == njt - 1),
                                    tile_position=(0, 0), skip_group_check=True)
                    # unnormalized values + denominator rows out of PSUM
                    otu = otupool.tile([128, IC_W], BF16, tag="otu", name="otu")
                    csrs = []
                    for h in range(2):
                        csr = csrpool.tile([1, IC_W], BF16, tag="csr",
                                           name="csr")
                        nc.vector.tensor_copy(csr[:], pso[h][64:65, :])
                        nc.vector.tensor_copy(
                            otu[64 * h:64 * h + 64, :], pso[h][0:64, :])
                        csrs.append(csr)
                    otus.append((otu, csrs))
                # ---- normalize + output projection for this chunk ----
                ots = []
                for p in range(2):
                    otu, csrs = otus[p]
                    dn = ps_m.tile([128, IC_W], F32, tag="m", name="dn")
                    nc.tensor.matmul(dn[0:64, :], onesr[:], csrs[0][:],
                                     start=True, stop=True)
                    nc.tensor.matmul(dn[64:128, :], onesr[:], csrs[1][:],
                                     start=True, stop=True)
                    rt = rtpool.tile([128, IC_W], F32, tag="rt", name="rt")
                    nc.vector.reciprocal_approx_fast(rt[:], dn[:])
                    ot = otpool.tile([128, IC_W], BF16, tag="ot", name="ot")
                    nc.vector.tensor_mul(ot[:], otu[:], rt[:])
                    ots.append(ot)
                for it in range(4):
                    osb = opool.tile([128, C], BF16, tag="osb", name="osb")
                    for ch in range(2):
                        psp = ps_m.tile([128, IC_W], F32, tag="m", name="psp")
                        nc.tensor.matmul(
                            psp[:], ots[0][:, 128 * it:128 * (it + 1)],
                            wo_sb[0][:, IC_W * ch:IC_W * (ch + 1)],
                            start=True, stop=False)
                        nc.tensor.matmul(
                            psp[:], ots[1][:, 128 * it:128 * (it + 1)],
                            wo_sb[1][:, IC_W * ch:IC_W * (ch + 1)],
                            start=False, stop=True)
                        nc.vector.tensor_copy(
                            osb[:, IC_W * ch:IC_W * (ch + 1)], psp[:])
                    r0 = i0 + 128 * it
                    nc.sync.dma_start(out_d.ap()[r0:r0 + 128, :], osb[:])
    nc.compile()
    return nc


_PROGS = {}


def _get_prog(causal: bool):
    if causal not in _PROGS:
        _PROGS[causal] = _build(causal)
    return _PROGS[causal]


def _prep_inputs(x, Wqkv, bqkv, Wo):
    """Per-core input maps (host-side sharding)."""
    x = np.asarray(x, dtype=np.float32)
    Wqkv = np.asarray(Wqkv, dtype=np.float32)
    bqkv = np.asarray(bqkv, dtype=np.float32)
    Wo = np.asarray(Wo, dtype=np.float32)

    scale = 1.0 / np.sqrt(np.float32(HS))

    jl = np.arange(JT)[:, None]
    il = np.arange(IC_W)[None, :]
    msk = np.stack([np.where(JT * r + jl <= il, 0.0, -1e9)
                    for r in range(4)]).astype(NPBF)
    idn = np.eye(128, dtype=NPBF)
    ones = np.ones((1, 64), dtype=NPBF)
    vni = np.zeros((128, 65 * N_JT), dtype=NPBF)
    vni[:, 64::65] = 1

    xts = [np.ascontiguousarray(x[b].T.astype(NPBF)) for b in range(B)]

    in_maps = []
    for c in range(N_CORES):
        b, g = c // 4, c % 4
        heads = [4 * g + k for k in range(4)]
        w3_cols, b3_rows = [], []
        for p in range(2):
            pair = heads[2 * p:2 * p + 2]
            for off, sc in ((0, scale), (HS, 1.0), (2 * HS, 1.0)):
                w3_cols.append(np.concatenate(
                    [Wqkv[:, 192 * h + off:192 * h + off + HS] * sc
                     for h in pair], axis=1))
                b3_rows.append(np.concatenate(
                    [bqkv[192 * h + off:192 * h + off + HS] * sc
                     for h in pair]))
        w3 = np.concatenate(w3_cols, axis=1).astype(NPBF)
        b3 = np.stack(b3_rows).astype(np.float32)[..., None]
        wo = np.concatenate([Wo[HS * h:HS * (h + 1), :] for h in heads],
                            axis=0).astype(NPBF)
        in_maps.append({
            "xt": xts[b],
            "w3": np.ascontiguousarray(w3),
            "b3": np.ascontiguousarray(b3),
            "wo": np.ascontiguousarray(wo),
            "msk": msk,
            "idn": idn,
            "ones": ones,
            "vni": vni,
        })
    return in_maps


class _Runner:
    """Cached shard_map runner for the SPMD NEFF (avoids re-jit per call)."""

    def __init__(self, nc):
        import jax
        from jax.sharding import Mesh, PartitionSpec
        from jax.experimental.shard_map import shard_map
        from concourse import bass2jax

        bass2jax.install_neuronx_cc_hook()

        part_name = (nc.partition_id_tensor.name
                     if nc.partition_id_tensor else None)
        in_names, out_names, out_avals, zero_outs = [], [], [], []
        for alloc in nc.m.functions[0].allocations:
            if not isinstance(alloc, mybir.MemoryLocationSet):
                continue
            name = alloc.memorylocations[0].name
            if alloc.kind == "ExternalInput":
                if name != part_name:
                    in_names.append(name)
            elif alloc.kind == "ExternalOutput":
                out_names.append(name)
                shape = tuple(alloc.tensor_shape)
                dtype = mybir.dt.np(alloc.dtype)
                out_avals.append(jax.core.ShapedArray(shape, dtype))
                zero_outs.append(np.zeros(shape, dtype))
        self.in_names, self.out_names = in_names, out_names
        self.zero_outs = zero_outs
        n_params, n_outs = len(in_names), len(out_names)
        all_in_names = tuple(in_names) + tuple(out_names)
        if part_name is not None:
            all_in_names = all_in_names + (part_name,)

        def _exec(args, outs):
            operands = list(args) + list(outs)
            if part_name is not None:
                operands.append(bass2jax.partition_id_tensor())
            return bass2jax._bass_exec_p.bind(
                *operands,
                out_avals=tuple(out_avals),
                in_names=all_in_names,
                out_names=tuple(out_names),
                lowering_input_output_aliases=(),
                sim_require_finite=True,
                sim_require_nnan=True,
                nc=nc)

        def _body(*args):
            ins, outs = args[:n_params], list(args[n_params:])
            return tuple(_exec(ins, outs))

        devices = jax.devices()[:N_CORES]
        mesh = Mesh(np.asarray(devices), ("core",))
        donate = tuple(range(n_params, n_params + n_outs))
        self._fn = jax.jit(
            shard_map(_body, mesh=mesh,
                      in_specs=(PartitionSpec("core"),) * (n_params + n_outs),
                      out_specs=(PartitionSpec("core"),) * n_outs,
                      check_rep=False),
            donate_argnums=donate, keep_unused=True)

    def __call__(self, in_maps):
        concat_in = [
            np.concatenate([in_maps[c][k] for c in range(N_CORES)], axis=0)
            for k in self.in_names]
        concat_zero = [
            np.zeros((N_CORES * z.shape[0], *z.shape[1:]), z.dtype)
            for z in self.zero_outs]
        out = self._fn(*concat_in, *concat_zero)
        return [
            {k: np.asarray(out[i]).reshape(N_CORES, *self.zero_outs[i].shape)[c]
             for i, k in enumerate(self.out_names)}
            for c in range(N_CORES)]


_RUNNERS = {}


def _get_runner(causal: bool):
    if causal not in _RUNNERS:
        _RUNNERS[causal] = _Runner(_get_prog(causal))
    return _RUNNERS[causal]


def kernel(x, Wqkv, bqkv, Wo, bo, mask):
    causal = bool(np.asarray(mask).item()) if not isinstance(mask, (int, bool)) \
        else bool(mask)
    runner = _get_runner(causal)
    in_maps = _prep_inputs(x, Wqkv, bqkv, Wo)
    results = runner(in_maps)
    out = np.zeros((B, T, C), dtype=np.float32)
    for c in range(N_CORES):
        out[c // 4] += results[c]["part"].astype(np.float32)
    out += np.asarray(bo, dtype=np.float32)[None, None, :]
    return out


# revision 22
# speedup vs baseline: 6.0934x; 1.0108x over previous
"""Multi-head self-attention (B=2, T=2048, C=1024, H=16) on 8 TRN2 NeuronCores.

Sharding: hybrid batch x head-group. Core c owns batch b = c//4 and heads
4g..4g+3 where g = c%4 (two head-pairs). Per core:
  - QKV projection for its batch rows and its 4 heads (bf16 matmuls,
    contraction-major x^T layout, biases folded in via tensor_scalar_add)
  - causal attention for its 4 heads, flash-style blockwise exp with no
    max-subtraction (scores are O(1)); causal mask applied as a -1e9 bias
    accumulated into the scores PSUM via an identity matmul, so exp gives
    exact zeros (no separate mask multiply)
  - softmax denominator via a ones-column in the value matrix (row 64 of
    the AV accumulation); per-i-chunk normalization with
    reciprocal_approx_fast + a selector-matmul broadcast
  - partial output projection partial = values @ Wo[rows of its heads]
Host sums the 4 partials per batch and adds bo.
"""

import numpy as np
import ml_dtypes

import concourse.bass as bass
import concourse.bacc as bacc
import concourse.mybir as mybir
import concourse.tile as tile
from concourse.bass_utils import run_bass_kernel_spmd

B, T, C = 2, 2048, 1024
H, HS = 16, 64
N_CORES = 8
IC_W = 512                     # i-chunk width (query cols per block)
N_IC = T // IC_W               # 4 i-chunks
JT = 128                       # j-tile width (key rows per block)
N_JT = T // JT                 # 16 j-tiles
BF16 = mybir.dt.bfloat16
F32 = mybir.dt.float32
NPBF = ml_dtypes.bfloat16


def _build(causal: bool):
    nc = bacc.Bacc("TRN2", target_bir_lowering=False, debug=False,
                   num_devices=N_CORES)

    xt_d = nc.dram_tensor("xt", [C, T], BF16, kind="ExternalInput")
    w3_d = nc.dram_tensor("w3", [C, 768], BF16, kind="ExternalInput")
    b3_d = nc.dram_tensor("b3", [6, 128, 1], F32, kind="ExternalInput")
    wo_d = nc.dram_tensor("wo", [256, C], BF16, kind="ExternalInput")
    msk_d = nc.dram_tensor("msk", [128, 128], BF16, kind="ExternalInput")
    one_d = nc.dram_tensor("ones", [1, 64], BF16, kind="ExternalInput")
    vni_d = nc.dram_tensor("vni", [128, 65 * N_JT], BF16, kind="ExternalInput")
    out_d = nc.dram_tensor("part", [T, C], BF16, kind="ExternalOutput")

    with tile.TileContext(nc) as tc:
        with (
            tc.tile_pool(name="const", bufs=1) as cpool,
            tc.tile_pool(name="xt", bufs=4) as xtpool,
            tc.tile_pool(name="pt", bufs=4) as ptpool,
            tc.tile_pool(name="otu", bufs=4) as otupool,
            tc.tile_pool(name="ot", bufs=4) as otpool,
            tc.tile_pool(name="csr", bufs=4) as csrpool,
            tc.tile_pool(name="rt", bufs=2) as rtpool,
            tc.tile_pool(name="osb", bufs=2) as opool,
            tc.tile_pool(name="ps_s", bufs=2, space="PSUM") as ps_s,
            tc.tile_pool(name="ps_o", bufs=2, space="PSUM") as ps_o,
            tc.tile_pool(name="ps_m", bufs=2, space="PSUM") as ps_m,
        ):
            # ---- constants (x chunks first so QKV can start ASAP) ----
            w3_sb = []
            for ct in range(8):
                t_ = cpool.tile([128, 768], BF16, tag=f"w3_{ct}", name=f"w3_{ct}")
                nc.sync.dma_start(t_[:], w3_d.ap()[128 * ct:128 * (ct + 1), :])
                w3_sb.append(t_)
            b3_sb = []
            for i in range(6):
                t_ = cpool.tile([128, 1], F32, tag=f"b3_{i}", name=f"b3_{i}")
                nc.sync.dma_start(t_[:], b3_d.ap()[i])
                b3_sb.append(t_)
            xts_all = []
            for icl in range(N_IC):
                i0 = IC_W * icl
                xts = []
                for ct in range(8):
                    t_ = xtpool.tile([128, IC_W], BF16, tag=f"x{ct}",
                                     name=f"x{ct}")
                    nc.sync.dma_start(
                        t_[:], xt_d.ap()[128 * ct:128 * (ct + 1), i0:i0 + IC_W])
                    xts.append(t_)
                xts_all.append(xts)
            wo_sb = []
            for p in range(2):
                t_ = cpool.tile([128, C], BF16, tag=f"wo_{p}", name=f"wo_{p}")
                nc.sync.dma_start(t_[:], wo_d.ap()[128 * p:128 * (p + 1), :])
                wo_sb.append(t_)
            msk01 = cpool.tile([128, 128], BF16, tag="msk01")
            nc.sync.dma_start(msk01[:], msk_d.ap()[:])
            onesr = cpool.tile([1, 64], BF16, tag="onesr")
            nc.sync.dma_start(onesr[:], one_d.ap()[:])
            vn_sb = []
            for h in range(4):
                t_ = cpool.tile([128, 65 * N_JT], BF16, tag=f"vn_{h}",
                                name=f"vn_{h}")
                nc.sync.dma_start(t_[:], vni_d.ap()[:])
                vn_sb.append(t_)

            qt2 = [cpool.tile([128, T], BF16, tag=f"qt2_{p}", name=f"qt2_{p}")
                   for p in range(2)]
            kt2 = [cpool.tile([128, T], BF16, tag=f"kt2_{p}", name=f"kt2_{p}")
                   for p in range(2)]
            vt2 = [cpool.tile([128, T], BF16, tag=f"vt2_{p}", name=f"vt2_{p}")
                   for p in range(2)]


            for icl in range(N_IC):
                i0 = IC_W * icl
                xts = xts_all[icl]
                # ---- QKV projection for this chunk ----
                for p in range(2):
                    for n, dsts in enumerate((qt2, kt2, vt2)):
                        c0 = 384 * p + 128 * n
                        ps = ps_m.tile([128, IC_W], F32, tag="m", name="ps_qkv")
                        for ct in range(8):
                            nc.tensor.matmul(
                                ps[:], w3_sb[ct][:, c0:c0 + 128], xts[ct][:],
                                start=(ct == 0), stop=(ct == 7))
                        nc.vector.tensor_scalar_add(
                            dsts[p][:, i0:i0 + IC_W], ps[:], b3_sb[3 * p + n][:])
                # ---- v natural layout (xbar DMA transpose, no PE/DVE) ----
                for p in range(2):
                    for jl in range(4):
                        jt = 4 * icl + jl
                        j0 = JT * jt
                        for h in range(2):
                            nc.sync.dma_start_transpose(
                                vn_sb[2 * p + h][:, 65 * jt:65 * jt + 64],
                                vt2[p][64 * h:64 * h + 64, j0:j0 + JT])
                # ---- attention for this chunk ----
                njt = 4 * (icl + 1) if causal else N_JT
                otus = []
                for p in range(2):
                    pso = [ps_o.tile([128, IC_W], F32, tag="o",
                                     name=f"pso{h_}") for h_ in range(2)]
                    for jtp in range(njt // 2):
                        pts = []
                        for h in range(2):
                            st = ps_s.tile([128, 2 * IC_W], F32, tag="s",
                                           name="st")
                            for half in range(2):
                                jt = 2 * jtp + half
                                j0 = JT * jt
                                c0 = IC_W * half
                                # diagonal tiles: only i >= 128*r_ is live
                                r_ = jt - 4 * icl if causal else -1
                                v0 = JT * r_ if r_ > 0 else 0
                                nc.tensor.matmul(
                                    st[:, c0 + v0:c0 + IC_W],
                                    kt2[p][64 * h:64 * h + 64, j0:j0 + JT],
                                    qt2[p][64 * h:64 * h + 64,
                                           i0 + v0:i0 + IC_W],
                                    start=True, stop=True,
                                    tile_position=(64 * h, 0))
                            pt = ptpool.tile([128, 2 * IC_W], BF16, tag="pt",
                                             name="pt")
                            if causal and 2 * jtp >= 4 * icl:
                                # diagonal pair: exp only the live staircase
                                for half in range(2):
                                    jt = 2 * jtp + half
                                    c0 = IC_W * half
                                    v0 = JT * (jt - 4 * icl)
                                    nc.scalar.activation(
                                        pt[:, c0 + v0:c0 + IC_W],
                                        st[:, c0 + v0:c0 + IC_W],
                                        mybir.ActivationFunctionType.Exp)
                                    # zero the masked triangle boundary block
                                    nc.vector.tensor_mul(
                                        pt[:, c0 + v0:c0 + v0 + JT],
                                        pt[:, c0 + v0:c0 + v0 + JT], msk01[:])
                            else:
                                nc.scalar.activation(
                                    pt[:], st[:],
                                    mybir.ActivationFunctionType.Exp)
                            pts.append(pt)
                        for h in range(2):
                            for half in range(2):
                                jt = 2 * jtp + half
                                c0 = IC_W * half
                                r_ = jt - 4 * icl if causal else -1
                                v0 = JT * r_ if r_ > 0 else 0
                                nc.tensor.matmul(
                                    pso[h][0:65, v0:IC_W],
                                    vn_sb[2 * p + h][:, 65 * jt:65 * jt + 65],
                                    pts[h][:, c0 + v0:c0 + IC_W],
                                    start=(jt == 0), stop=(jt == njt - 1),
                                    tile_position=(0, 0), skip_group_check=True)
                    # unnormalized values + denominator rows out of PSUM
                    otu = otupool.tile([128, IC_W], BF16, tag="otu", name="otu")
                    csrs = []
                    for h in range(2):
                        csr = csrpool.tile([1, IC_W], BF16, tag="csr",
                                           name="csr")
                        nc.vector.tensor_copy(csr[:], pso[h][64:65, :])
                        nc.vector.tensor_copy(
                            otu[64 * h:64 * h + 64, :], pso[h][0:64, :])
                        csrs.append(csr)
                    otus.append((otu, csrs))
                # ---- normalize + output projection for this chunk ----
                ots = []
                for p in range(2):
                    otu, csrs = otus[p]
                    dn = ps_m.tile([128, IC_W], F32, tag="m", name="dn")
                    nc.tensor.matmul(dn[0:64, :], onesr[:], csrs[0][:],
                                     start=True, stop=True)
                    nc.tensor.matmul(dn[64:128, :], onesr[:], csrs[1][:],
                                     start=True, stop=True)
                    rt = rtpool.tile([128, IC_W], F32, tag="rt", name="rt")
                    nc.vector.reciprocal_approx_fast(rt[:], dn[:])
                    ot = otpool.tile([128, IC_W], BF16, tag="ot", name="ot")
                    nc.vector.tensor_mul(ot[:], otu[:], rt[:])
                    ots.append(ot)
                for it in range(4):
                    osb = opool.tile([128, C], BF16, tag="osb", name="osb")
                    for ch in range(2):
                        psp = ps_m.tile([128, IC_W], F32, tag="m", name="psp")
                        nc.tensor.matmul(
                            psp[:], ots[0][:, 128 * it:128 * (it + 1)],
                            wo_sb[0][:, IC_W * ch:IC_W * (ch + 1)],
                            start=True, stop=False)
                        nc.tensor.matmul(
                            psp[:], ots[1][:, 128 * it:128 * (it + 1)],
                            wo_sb[1][:, IC_W * ch:IC_W * (ch + 1)],
                            start=False, stop=True)
                        nc.vector.tensor_copy(
                            osb[:, IC_W * ch:IC_W * (ch + 1)], psp[:])
                    r0 = i0 + 128 * it
                    nc.sync.dma_start(out_d.ap()[r0:r0 + 128, :], osb[:])
    nc.compile()
    return nc


_PROGS = {}


def _get_prog(causal: bool):
    if causal not in _PROGS:
        _PROGS[causal] = _build(causal)
    return _PROGS[causal]


def _prep_inputs(x, Wqkv, bqkv, Wo):
    """Per-core input maps (host-side sharding)."""
    x = np.asarray(x, dtype=np.float32)
    Wqkv = np.asarray(Wqkv, dtype=np.float32)
    bqkv = np.asarray(bqkv, dtype=np.float32)
    Wo = np.asarray(Wo, dtype=np.float32)

    scale = 1.0 / np.sqrt(np.float32(HS))

    jl = np.arange(JT)[:, None]
    il = np.arange(JT)[None, :]
    msk = np.where(jl <= il, 1.0, 0.0).astype(NPBF)
    ones = np.ones((1, 64), dtype=NPBF)
    vni = np.zeros((128, 65 * N_JT), dtype=NPBF)
    vni[:, 64::65] = 1

    xts = [np.ascontiguousarray(x[b].T.astype(NPBF)) for b in range(B)]

    in_maps = []
    for c in range(N_CORES):
        b, g = c // 4, c % 4
        heads = [4 * g + k for k in range(4)]
        w3_cols, b3_rows = [], []
        for p in range(2):
            pair = heads[2 * p:2 * p + 2]
            for off, sc in ((0, scale), (HS, 1.0), (2 * HS, 1.0)):
                w3_cols.append(np.concatenate(
                    [Wqkv[:, 192 * h + off:192 * h + off + HS] * sc
                     for h in pair], axis=1))
                b3_rows.append(np.concatenate(
                    [bqkv[192 * h + off:192 * h + off + HS] * sc
                     for h in pair]))
        w3 = np.concatenate(w3_cols, axis=1).astype(NPBF)
        b3 = np.stack(b3_rows).astype(np.float32)[..., None]
        wo = np.concatenate([Wo[HS * h:HS * (h + 1), :] for h in heads],
                            axis=0).astype(NPBF)
        in_maps.append({
            "xt": xts[b],
            "w3": np.ascontiguousarray(w3),
            "b3": np.ascontiguousarray(b3),
            "wo": np.ascontiguousarray(wo),
            "msk": msk,
            "ones": ones,
            "vni": vni,
        })
    return in_maps


class _Runner:
    """Cached shard_map runner for the SPMD NEFF (avoids re-jit per call)."""

    def __init__(self, nc):
        import jax
        from jax.sharding import Mesh, PartitionSpec
        from jax.experimental.shard_map import shard_map
        from concourse import bass2jax

        bass2jax.install_neuronx_cc_hook()

        part_name = (nc.partition_id_tensor.name
                     if nc.partition_id_tensor else None)
        in_names, out_names, out_avals, zero_outs = [], [], [], []
        for alloc in nc.m.functions[0].allocations:
            if not isinstance(alloc, mybir.MemoryLocationSet):
                continue
            name = alloc.memorylocations[0].name
            if alloc.kind == "ExternalInput":
                if name != part_name:
                    in_names.append(name)
            elif alloc.kind == "ExternalOutput":
                out_names.append(name)
                shape = tuple(alloc.tensor_shape)
                dtype = mybir.dt.np(alloc.dtype)
                out_avals.append(jax.core.ShapedArray(shape, dtype))
                zero_outs.append(np.zeros(shape, dtype))
        self.in_names, self.out_names = in_names, out_names
        self.zero_outs = zero_outs
        n_params, n_outs = len(in_names), len(out_names)
        all_in_names = tuple(in_names) + tuple(out_names)
        if part_name is not None:
            all_in_names = all_in_names + (part_name,)

        def _exec(args, outs):
            operands = list(args) + list(outs)
            if part_name is not None:
                operands.append(bass2jax.partition_id_tensor())
            return bass2jax._bass_exec_p.bind(
                *operands,
                out_avals=tuple(out_avals),
                in_names=all_in_names,
                out_names=tuple(out_names),
                lowering_input_output_aliases=(),
                sim_require_finite=True,
                sim_require_nnan=True,
                nc=nc)

        def _body(*args):
            ins, outs = args[:n_params], list(args[n_params:])
            return tuple(_exec(ins, outs))

        devices = jax.devices()[:N_CORES]
        mesh = Mesh(np.asarray(devices), ("core",))
        donate = tuple(range(n_params, n_params + n_outs))
        self._fn = jax.jit(
            shard_map(_body, mesh=mesh,
                      in_specs=(PartitionSpec("core"),) * (n_params + n_outs),
                      out_specs=(PartitionSpec("core"),) * n_outs,
                      check_rep=False),
            donate_argnums=donate, keep_unused=True)

    def __call__(self, in_maps):
        concat_in = [
            np.concatenate([in_maps[c][k] for c in range(N_CORES)], axis=0)
            for k in self.in_names]
        concat_zero = [
            np.zeros((N_CORES * z.shape[0], *z.shape[1:]), z.dtype)
            for z in self.zero_outs]
        out = self._fn(*concat_in, *concat_zero)
        return [
            {k: np.asarray(out[i]).reshape(N_CORES, *self.zero_outs[i].shape)[c]
             for i, k in enumerate(self.out_names)}
            for c in range(N_CORES)]


_RUNNERS = {}


def _get_runner(causal: bool):
    if causal not in _RUNNERS:
        _RUNNERS[causal] = _Runner(_get_prog(causal))
    return _RUNNERS[causal]


def kernel(x, Wqkv, bqkv, Wo, bo, mask):
    causal = bool(np.asarray(mask).item()) if not isinstance(mask, (int, bool)) \
        else bool(mask)
    runner = _get_runner(causal)
    in_maps = _prep_inputs(x, Wqkv, bqkv, Wo)
    results = runner(in_maps)
    out = np.zeros((B, T, C), dtype=np.float32)
    for c in range(N_CORES):
        out[c // 4] += results[c]["part"].astype(np.float32)
    out += np.asarray(bo, dtype=np.float32)[None, None, :]
    return out


# revision 29
# speedup vs baseline: 6.4506x; 1.0586x over previous
"""Multi-head self-attention (B=2, T=2048, C=1024, H=16) on 8 TRN2 NeuronCores.

Sharding: hybrid batch x head-group. Core c owns batch b = c//4 and heads
4g..4g+3 where g = c%4 (two head-pairs). Per core:
  - QKV projection for its batch rows and its 4 heads (bf16 matmuls,
    contraction-major x^T layout, biases folded in via tensor_scalar_add)
  - causal attention for its 4 heads, flash-style blockwise exp with no
    max-subtraction (scores are O(1)); causal mask applied as a -1e9 bias
    accumulated into the scores PSUM via an identity matmul, so exp gives
    exact zeros (no separate mask multiply)
  - softmax denominator via a ones-column in the value matrix (row 64 of
    the AV accumulation); per-i-chunk normalization with
    reciprocal_approx_fast + a selector-matmul broadcast
  - partial output projection partial = values @ Wo[rows of its heads]
Host sums the 4 partials per batch and adds bo.
"""

import numpy as np
import ml_dtypes

import concourse.bass as bass
import concourse.bacc as bacc
import concourse.mybir as mybir
import concourse.tile as tile
from concourse.bass_utils import run_bass_kernel_spmd

B, T, C = 2, 2048, 1024
H, HS = 16, 64
N_CORES = 8
IC_W = 512                     # i-chunk width (query cols per block)
N_IC = T // IC_W               # 4 i-chunks
JT = 128                       # j-tile width (key rows per block)
N_JT = T // JT                 # 16 j-tiles
BF16 = mybir.dt.bfloat16
F32 = mybir.dt.float32
NPBF = ml_dtypes.bfloat16
USE_DMA_T = False


def _build(causal: bool):
    nc = bacc.Bacc("TRN2", target_bir_lowering=False, debug=False,
                   num_devices=N_CORES)

    xt_d = nc.dram_tensor("xt", [C, T], BF16, kind="ExternalInput")
    w3_d = nc.dram_tensor("w3", [C, 768], BF16, kind="ExternalInput")
    b3_d = nc.dram_tensor("b3", [6, 128, 1], F32, kind="ExternalInput")
    wo_d = nc.dram_tensor("wo", [256, C], BF16, kind="ExternalInput")
    msk_d = nc.dram_tensor("msk", [128, 128], BF16, kind="ExternalInput")
    idn_d = nc.dram_tensor("idn", [128, 128], BF16, kind="ExternalInput")
    one_d = nc.dram_tensor("ones", [1, 64], BF16, kind="ExternalInput")
    vni_d = nc.dram_tensor("vni", [128, 65 * N_JT], BF16, kind="ExternalInput")
    out_d = nc.dram_tensor("part", [T, C], BF16, kind="ExternalOutput")

    with tile.TileContext(nc) as tc:
        with (
            tc.tile_pool(name="const", bufs=1) as cpool,
            tc.tile_pool(name="xt", bufs=4) as xtpool,
            tc.tile_pool(name="pt", bufs=4) as ptpool,
            tc.tile_pool(name="otu", bufs=4) as otupool,
            tc.tile_pool(name="ot", bufs=4) as otpool,
            tc.tile_pool(name="csr", bufs=4) as csrpool,
            tc.tile_pool(name="rt", bufs=2) as rtpool,
            tc.tile_pool(name="osb", bufs=2) as opool,
            tc.tile_pool(name="ps_s", bufs=2, space="PSUM") as ps_s,
            tc.tile_pool(name="ps_o", bufs=2, space="PSUM") as ps_o,
            tc.tile_pool(name="ps_m", bufs=2, space="PSUM") as ps_m,
        ):
            # ---- constants (x chunks first so QKV can start ASAP) ----
            w3_sb = []
            for ct in range(8):
                t_ = cpool.tile([128, 768], BF16, tag=f"w3_{ct}", name=f"w3_{ct}")
                nc.sync.dma_start(t_[:], w3_d.ap()[128 * ct:128 * (ct + 1), :])
                w3_sb.append(t_)
            b3_sb = []
            for i in range(6):
                t_ = cpool.tile([128, 1], F32, tag=f"b3_{i}", name=f"b3_{i}")
                nc.sync.dma_start(t_[:], b3_d.ap()[i])
                b3_sb.append(t_)
            xts_all = []
            for icl in range(N_IC):
                i0 = IC_W * icl
                xts = []
                for ct in range(8):
                    t_ = xtpool.tile([128, IC_W], BF16, tag=f"x{ct}",
                                     name=f"x{ct}")
                    nc.sync.dma_start(
                        t_[:], xt_d.ap()[128 * ct:128 * (ct + 1), i0:i0 + IC_W])
                    xts.append(t_)
                xts_all.append(xts)
            wo_sb = []
            for p in range(2):
                t_ = cpool.tile([128, C], BF16, tag=f"wo_{p}", name=f"wo_{p}")
                nc.sync.dma_start(t_[:], wo_d.ap()[128 * p:128 * (p + 1), :])
                wo_sb.append(t_)
            msk01 = cpool.tile([128, 128], BF16, tag="msk01")
            nc.sync.dma_start(msk01[:], msk_d.ap()[:])
            idn_sb = cpool.tile([128, 128], BF16, tag="idn")
            nc.sync.dma_start(idn_sb[:], idn_d.ap()[:])
            onesr = cpool.tile([1, 64], BF16, tag="onesr")
            nc.sync.dma_start(onesr[:], one_d.ap()[:])
            vn_sb = []
            for h in range(4):
                t_ = cpool.tile([128, 65 * N_JT], BF16, tag=f"vn_{h}",
                                name=f"vn_{h}")
                nc.sync.dma_start(t_[:], vni_d.ap()[:])
                vn_sb.append(t_)

            qt2 = [cpool.tile([128, T], BF16, tag=f"qt2_{p}", name=f"qt2_{p}")
                   for p in range(2)]
            kt2 = [cpool.tile([128, T], BF16, tag=f"kt2_{p}", name=f"kt2_{p}")
                   for p in range(2)]
            vt2 = [cpool.tile([128, T], BF16, tag=f"vt2_{p}", name=f"vt2_{p}")
                   for p in range(2)]


            for icl in range(N_IC):
                i0 = IC_W * icl
                xts = xts_all[icl]
                # ---- QKV projection for this chunk ----
                for p in range(2):
                    for n, dsts in enumerate((qt2, kt2, vt2)):
                        c0 = 384 * p + 128 * n
                        ps = ps_m.tile([128, IC_W], F32, tag="m", name="ps_qkv")
                        for ct in range(8):
                            nc.tensor.matmul(
                                ps[:], w3_sb[ct][:, c0:c0 + 128], xts[ct][:],
                                start=(ct == 0), stop=(ct == 7))
                        nc.vector.tensor_scalar_add(
                            dsts[p][:, i0:i0 + IC_W], ps[:], b3_sb[3 * p + n][:])
                # ---- v natural layout (transpose this chunk's j-tiles) ----
                if USE_DMA_T:
                    for p in range(2):
                        for jl in range(4):
                            jt = 4 * icl + jl
                            j0 = JT * jt
                            for h in range(2):
                                nc.sync.dma_start_transpose(
                                    vn_sb[2 * p + h][:, 65 * jt:65 * jt + 64],
                                    vt2[p][64 * h:64 * h + 64, j0:j0 + JT])
                else:
                    for p in range(2):
                        for jl in range(4):
                            jt = 4 * icl + jl
                            j0 = JT * jt
                            psv = ps_m.tile([128, JT], BF16, tag="m",
                                            name="psv")
                            nc.tensor.transpose(
                                psv[:, 0:JT], vt2[p][:, j0:j0 + JT], idn_sb[:])
                            nc.vector.tensor_copy(
                                vn_sb[2 * p][:, 65 * jt:65 * jt + 64],
                                psv[:, 0:64])
                            nc.vector.tensor_copy(
                                vn_sb[2 * p + 1][:, 65 * jt:65 * jt + 64],
                                psv[:, 64:128])
                # ---- attention for this chunk ----
                njt = 4 * (icl + 1) if causal else N_JT
                otus = []
                for p in range(2):
                    pso = [ps_o.tile([128, IC_W], F32, tag="o",
                                     name=f"pso{h_}") for h_ in range(2)]
                    for jtp in range(njt // 2):
                        pts = []
                        for h in range(2):
                            st = ps_s.tile([128, 2 * IC_W], F32, tag="s",
                                           name="st")
                            for half in range(2):
                                jt = 2 * jtp + half
                                j0 = JT * jt
                                c0 = IC_W * half
                                # diagonal tiles: only i >= 128*r_ is live
                                r_ = jt - 4 * icl if causal else -1
                                v0 = JT * r_ if r_ > 0 else 0
                                nc.tensor.matmul(
                                    st[:, c0 + v0:c0 + IC_W],
                                    kt2[p][64 * h:64 * h + 64, j0:j0 + JT],
                                    qt2[p][64 * h:64 * h + 64,
                                           i0 + v0:i0 + IC_W],
                                    start=True, stop=True,
                                    tile_position=(64 * h, 0))
                            pt = ptpool.tile([128, 2 * IC_W], BF16, tag="pt",
                                             name="pt")
                            if causal and 2 * jtp >= 4 * icl:
                                # diagonal pair: exp only the live staircase
                                for half in range(2):
                                    jt = 2 * jtp + half
                                    c0 = IC_W * half
                                    v0 = JT * (jt - 4 * icl)
                                    nc.scalar.activation(
                                        pt[:, c0 + v0:c0 + IC_W],
                                        st[:, c0 + v0:c0 + IC_W],
                                        mybir.ActivationFunctionType.Exp)
                                    # zero the masked triangle boundary block
                                    nc.vector.tensor_mul(
                                        pt[:, c0 + v0:c0 + v0 + JT],
                                        pt[:, c0 + v0:c0 + v0 + JT], msk01[:])
                            else:
                                nc.scalar.activation(
                                    pt[:], st[:],
                                    mybir.ActivationFunctionType.Exp)
                            pts.append(pt)
                        for h in range(2):
                            for half in range(2):
                                jt = 2 * jtp + half
                                c0 = IC_W * half
                                r_ = jt - 4 * icl if causal else -1
                                v0 = JT * r_ if r_ > 0 else 0
                                nc.tensor.matmul(
                                    pso[h][0:65, v0:IC_W],
                                    vn_sb[2 * p + h][:, 65 * jt:65 * jt + 65],
                                    pts[h][:, c0 + v0:c0 + IC_W],
                                    start=(jt == 0), stop=(jt == njt - 1),
                                    tile_position=(0, 0), skip_group_check=True)
                    # unnormalized values + denominator rows out of PSUM
                    otu = otupool.tile([128, IC_W], BF16, tag="otu", name="otu")
                    csrs = []
                    for h in range(2):
                        csr = csrpool.tile([1, IC_W], BF16, tag="csr",
                                           name="csr")
                        nc.vector.tensor_copy(csr[:], pso[h][64:65, :])
                        nc.vector.tensor_copy(
                            otu[64 * h:64 * h + 64, :], pso[h][0:64, :])
                        csrs.append(csr)
                    otus.append((otu, csrs))
                # ---- normalize + output projection for this chunk ----
                ots = []
                for p in range(2):
                    otu, csrs = otus[p]
                    dn = ps_m.tile([128, IC_W], F32, tag="m", name="dn")
                    nc.tensor.matmul(dn[0:64, :], onesr[:], csrs[0][:],
                                     start=True, stop=True)
                    nc.tensor.matmul(dn[64:128, :], onesr[:], csrs[1][:],
                                     start=True, stop=True)
                    rt = rtpool.tile([128, IC_W], F32, tag="rt", name="rt")
                    nc.vector.reciprocal_approx_fast(rt[:], dn[:])
                    ot = otpool.tile([128, IC_W], BF16, tag="ot", name="ot")
                    nc.vector.tensor_mul(ot[:], otu[:], rt[:])
                    ots.append(ot)
                for it in range(4):
                    osb = opool.tile([128, C], BF16, tag="osb", name="osb")
                    for ch in range(2):
                        psp = ps_m.tile([128, IC_W], F32, tag="m", name="psp")
                        nc.tensor.matmul(
                            psp[:], ots[0][:, 128 * it:128 * (it + 1)],
                            wo_sb[0][:, IC_W * ch:IC_W * (ch + 1)],
                            start=True, stop=False)
                        nc.tensor.matmul(
                            psp[:], ots[1][:, 128 * it:128 * (it + 1)],
                            wo_sb[1][:, IC_W * ch:IC_W * (ch + 1)],
                            start=False, stop=True)
                        nc.vector.tensor_copy(
                            osb[:, IC_W * ch:IC_W * (ch + 1)], psp[:])
                    r0 = i0 + 128 * it
                    nc.sync.dma_start(out_d.ap()[r0:r0 + 128, :], osb[:])
    nc.compile()
    return nc


_PROGS = {}


def _get_prog(causal: bool):
    if causal not in _PROGS:
        _PROGS[causal] = _build(causal)
    return _PROGS[causal]


def _prep_inputs(x, Wqkv, bqkv, Wo):
    """Per-core input maps (host-side sharding)."""
    x = np.asarray(x, dtype=np.float32)
    Wqkv = np.asarray(Wqkv, dtype=np.float32)
    bqkv = np.asarray(bqkv, dtype=np.float32)
    Wo = np.asarray(Wo, dtype=np.float32)

    scale = 1.0 / np.sqrt(np.float32(HS))

    jl = np.arange(JT)[:, None]
    il = np.arange(JT)[None, :]
    msk = np.where(jl <= il, 1.0, 0.0).astype(NPBF)
    idn = np.eye(128, dtype=NPBF)
    ones = np.ones((1, 64), dtype=NPBF)
    vni = np.zeros((128, 65 * N_JT), dtype=NPBF)
    vni[:, 64::65] = 1

    xts = [np.ascontiguousarray(x[b].T.astype(NPBF)) for b in range(B)]

    in_maps = []
    for c in range(N_CORES):
        b, g = c // 4, c % 4
        heads = [4 * g + k for k in range(4)]
        w3_cols, b3_rows = [], []
        for p in range(2):
            pair = heads[2 * p:2 * p + 2]
            for off, sc in ((0, scale), (HS, 1.0), (2 * HS, 1.0)):
                w3_cols.append(np.concatenate(
                    [Wqkv[:, 192 * h + off:192 * h + off + HS] * sc
                     for h in pair], axis=1))
                b3_rows.append(np.concatenate(
                    [bqkv[192 * h + off:192 * h + off + HS] * sc
                     for h in pair]))
        w3 = np.concatenate(w3_cols, axis=1).astype(NPBF)
        b3 = np.stack(b3_rows).astype(np.float32)[..., None]
        wo = np.concatenate([Wo[HS * h:HS * (h + 1), :] for h in heads],
                            axis=0).astype(NPBF)
        in_maps.append({
            "xt": xts[b],
            "w3": np.ascontiguousarray(w3),
            "b3": np.ascontiguousarray(b3),
            "wo": np.ascontiguousarray(wo),
            "msk": msk,
            "idn": idn,
            "ones": ones,
            "vni": vni,
        })
    return in_maps


class _Runner:
    """Cached shard_map runner for the SPMD NEFF (avoids re-jit per call)."""

    def __init__(self, nc):
        import jax
        from jax.sharding import Mesh, PartitionSpec
        from jax.experimental.shard_map import shard_map
        from concourse import bass2jax

        bass2jax.install_neuronx_cc_hook()

        part_name = (nc.partition_id_tensor.name
                     if nc.partition_id_tensor else None)
        in_names, out_names, out_avals, zero_outs = [], [], [], []
        for alloc in nc.m.functions[0].allocations:
            if not isinstance(alloc, mybir.MemoryLocationSet):
                continue
            name = alloc.memorylocations[0].name
            if alloc.kind == "ExternalInput":
                if name != part_name:
                    in_names.append(name)
            elif alloc.kind == "ExternalOutput":
                out_names.append(name)
                shape = tuple(alloc.tensor_shape)
                dtype = mybir.dt.np(alloc.dtype)
                out_avals.append(jax.core.ShapedArray(shape, dtype))
                zero_outs.append(np.zeros(shape, dtype))
        self.in_names, self.out_names = in_names, out_names
        self.zero_outs = zero_outs
        n_params, n_outs = len(in_names), len(out_names)
        all_in_names = tuple(in_names) + tuple(out_names)
        if part_name is not None:
            all_in_names = all_in_names + (part_name,)

        def _exec(args, outs):
            operands = list(args) + list(outs)
            if part_name is not None:
                operands.append(bass2jax.partition_id_tensor())
            return bass2jax._bass_exec_p.bind(
                *operands,
                out_avals=tuple(out_avals),
                in_names=all_in_names,
                out_names=tuple(out_names),
                lowering_input_output_aliases=(),
                sim_require_finite=True,
                sim_require_nnan=True,
                nc=nc)

        def _body(*args):
            ins, outs = args[:n_params], list(args[n_params:])
            return tuple(_exec(ins, outs))

        devices = jax.devices()[:N_CORES]
        mesh = Mesh(np.asarray(devices), ("core",))
        donate = tuple(range(n_params, n_params + n_outs))
        self._fn = jax.jit(
            shard_map(_body, mesh=mesh,
                      in_specs=(PartitionSpec("core"),) * (n_params + n_outs),
                      out_specs=(PartitionSpec("core"),) * n_outs,
                      check_rep=False),
            donate_argnums=donate, keep_unused=True)

    def __call__(self, in_maps):
        concat_in = [
            np.concatenate([in_maps[c][k] for c in range(N_CORES)], axis=0)
            for k in self.in_names]
        concat_zero = [
            np.zeros((N_CORES * z.shape[0], *z.shape[1:]), z.dtype)
            for z in self.zero_outs]
        out = self._fn(*concat_in, *concat_zero)
        return [
            {k: np.asarray(out[i]).reshape(N_CORES, *self.zero_outs[i].shape)[c]
             for i, k in enumerate(self.out_names)}
            for c in range(N_CORES)]


_RUNNERS = {}


def _get_runner(causal: bool):
    if causal not in _RUNNERS:
        _RUNNERS[causal] = _Runner(_get_prog(causal))
    return _RUNNERS[causal]


def kernel(x, Wqkv, bqkv, Wo, bo, mask):
    causal = bool(np.asarray(mask).item()) if not isinstance(mask, (int, bool)) \
        else bool(mask)
    runner = _get_runner(causal)
    in_maps = _prep_inputs(x, Wqkv, bqkv, Wo)
    results = runner(in_maps)
    out = np.zeros((B, T, C), dtype=np.float32)
    for c in range(N_CORES):
        out[c // 4] += results[c]["part"].astype(np.float32)
    out += np.asarray(bo, dtype=np.float32)[None, None, :]
    return out
